# revision 13
# baseline (speedup 1.0000x reference)
"""Trainium2 Bass kernel for nn_Attention_77446850281941.

Computes, for dec_hidden [32,1024], enc_outputs [2048,32,1024], W [1,2048], b [1]:
    e[b,s]  = dec_hidden[b]@W[0,:1024] + enc_outputs[s,b,:]@W[0,1024:] + b[0]
    out     = softmax(tanh(e), axis=s)            -> [32, 2048] float32

Sharding: batch (32) is split across 8 NeuronCores (4 rows each); W/b are
replicated. Softmax rows live entirely on one core, so no collectives.

Per-core dataflow: the enc shard [2048, 4, 1024] f32 (32 MB) streams in
s-chunks of 128 (partition = s, free = (b, e); 16 KB contiguous per
partition per DMA).  The weighted e-reduction (sum_e enc*w_enc) is spread
across three engines so it stays under the ~5.6 us/slab DMA stream time
(the previous all-DVE version was VectorE-bound at 1.47 us per column):
 - b=0: GpSimd fused scalar_tensor_tensor (mult + free-axis accumulate).
 - b=1..3: DVE plain tensor_mul into a scratch product (1x f32 rate),
   then ScalarE Identity-activation with accum_out does the reduce.
 - DVE adds the per-b dec_hidden-dot + bias, ScalarE applies tanh then
   exp per chunk.  tanh output is in [-1,1] so exp needs no max shift.
 - Row sums cross partitions via a PE ones-matmul; the final [128, 64]
   tile is PE-transposed so the output DMA writes contiguous 512B rows.
"""

import sys

import numpy as np

for _p in ("/opt/trn_rl_repo",):
    if _p not in sys.path:
        sys.path.insert(0, _p)

import concourse.bacc as bacc
import concourse.tile as tile
from concourse import mybir
from concourse.bass_utils import run_bass_kernel_spmd

F32 = mybir.dt.float32
SRC = 2048          # src_len
BATCH = 32
EH2 = 1024          # 2*enc_hid_dim
DH = 1024           # dec_hid_dim
NCORES = 8
BPC = BATCH // NCORES      # batch rows per core = 4
NCHUNK = SRC // 128        # s-chunks per core = 16
SLAB_BUFS = 6
SPLIT_FIRST = 2            # how many leading slabs get per-b sub-DMAs
PROD_BUFS = 3              # scratch product tiles (mult write -> reduce read)

_NC_CACHE = {}


def build_nc():
    nc = bacc.Bacc("TRN2", target_bir_lowering=False, debug=False)

    enc = nc.dram_tensor("enc", [SRC, BPC, EH2], F32, kind="ExternalInput").ap()
    # w_enc pre-replicated host-side: [128, BPC-1, EH2] (same row everywhere)
    w3 = nc.dram_tensor("w3", [128, BPC - 1, EH2], F32,
                        kind="ExternalInput").ap()
    # dec_bc[p, b] = dec_hidden[b]·w_dec + bias (same for every partition p)
    dbc = nc.dram_tensor("dbc", [128, BPC], F32, kind="ExternalInput").ap()
    # [:, :128] identity; [0:BPC, 128:192] G4 with G4[b, m] = (m//16 == b)
    ident = nc.dram_tensor("ident", [128, 192], F32, kind="ExternalInput").ap()
    out = nc.dram_tensor("out", [BPC * NCHUNK, 128], F32, kind="ExternalOutput").ap()

    MUL = mybir.AluOpType.mult
    ADD = mybir.AluOpType.add
    ACT = mybir.ActivationFunctionType

    with tile.TileContext(nc) as tc:
        with (
            tc.tile_pool(name="consts", bufs=1) as consts,
            tc.tile_pool(name="slabs", bufs=SLAB_BUFS) as slabs,
            tc.tile_pool(name="firsts", bufs=BPC * SPLIT_FIRST) as firsts,
            tc.tile_pool(name="prods", bufs=PROD_BUFS) as prods,
            tc.tile_pool(name="acc", bufs=1) as acc,
            tc.tile_pool(name="small", bufs=1) as small,
            tc.tile_pool(name="psum", bufs=1, space="PSUM") as psum,
        ):
            # consts ride the scalar HWDGE ring so the sync ring is free for
            # the enc slab stream; the 1.5 MB w3 overlaps the first slabs
            w3_sb = consts.tile([128, BPC - 1, EH2], F32)
            nc.scalar.dma_start(out=w3_sb, in_=w3)
            w_sb = w3_sb[:, 0, :]
            dec_bc = consts.tile([128, BPC], F32)
            nc.scalar.dma_start(out=dec_bc, in_=dbc)
            id_sb = consts.tile([128, 192], F32)
            nc.scalar.dma_start(out=id_sb, in_=ident)
            onec_sb = consts.tile([128, 1], F32)
            nc.gpsimd.memset(onec_sb, 1.0)

            # stride-0 dump columns for unused full elementwise results
            dump_g = small.tile([128, 1], F32)
            dump_a = small.tile([128, 1], F32)

            # e_cols[p, b, t] = enc[t*128+p, b, :]·w_enc;  exp_t = exp(tanh(.))
            e_cols = acc.tile([128, BPC, NCHUNK], F32)
            texp = acc.tile([128, BPC, NCHUNK], F32)
            exp_t = acc.tile([128, BPC, NCHUNK], F32)
            for t in range(NCHUNK):
                if t < SPLIT_FIRST:
                    # split the first slab(s) so compute starts after 512 KB
                    parts = []
                    for b_ in range(BPC):
                        sub = firsts.tile([128, EH2], F32, tag="first")
                        nc.sync.dma_start(
                            out=sub, in_=enc[t * 128:(t + 1) * 128, b_, :])
                        parts.append(sub)
                    bslice = lambda b_: parts[b_]
                else:
                    slab = slabs.tile([128, BPC, EH2], F32)
                    nc.sync.dma_start(
                        out=slab, in_=enc[t * 128:(t + 1) * 128, :, :])
                    bslice = lambda b_: slab[:, b_, :]
                # b=1..3: DVE multiply (one wide op), ScalarE accum-reduce.
                # The wide mult goes first so ScalarE unblocks earliest.
                prod = prods.tile([128, BPC - 1, EH2], F32, tag="prod")
                if t < SPLIT_FIRST:
                    for b_ in range(1, BPC):
                        nc.vector.tensor_mul(
                            prod[:, b_ - 1, :], bslice(b_), w_sb)
                else:
                    nc.vector.tensor_mul(prod, slab[:, 1:BPC, :], w3_sb)
                # b=0: DVE fused multiply + free-axis accumulate
                nc.vector.scalar_tensor_tensor(
                    out=dump_g.broadcast_to((128, EH2)),
                    in0=bslice(0), scalar=1.0, in1=w_sb,
                    op0=MUL, op1=MUL, accum_out=e_cols[:, 0, t:t + 1])
                for b_ in range(1, BPC):
                    nc.scalar.activation(
                        out=dump_a.broadcast_to((128, EH2)),
                        in_=prod[:, b_ - 1, :], func=ACT.Identity,
                        accum_out=e_cols[:, b_, t:t + 1])

            # bulk epilogue: tanh(e + dec·w_dec + bias) via the per-partition
            # bias port (dec_bc[p, b] is constant over p), then exp over all
            # 64 (b, t) columns at once -- keeps the slab loop free of the
            # per-chunk DVE<->ScalarE ping-pong.
            for b_ in range(BPC):
                nc.scalar.activation(
                    out=texp[:, b_, :], in_=e_cols[:, b_, :], func=ACT.Tanh,
                    bias=dec_bc[:, b_:b_ + 1], scale=1.0)
            nc.scalar.activation(out=exp_t, in_=texp, func=ACT.Exp)

            # transpose unnormalized exp: [128, (b,t)] -> [(b,t), 128]
            # (runs on PE/ACT in parallel with the denominator chain below)
            p_out = psum.tile([BPC * NCHUNK, 128], F32)
            nc.tensor.transpose(p_out, exp_t[:, :, :], id_sb[:, 0:128])
            out_unn = small.tile([BPC * NCHUNK, 128], F32)
            nc.scalar.activation(out=out_unn, in_=p_out, func=ACT.Identity)

            # denominator: per-b sum over t (DVE) then s (PE), as a column
            sums = small.tile([128, BPC], F32)
            nc.vector.tensor_reduce(
                out=sums, in_=exp_t[:, :, :],
                axis=mybir.AxisListType.X, op=ADD)
            p_tot = psum.tile([BPC, 1], F32)
            nc.tensor.matmul(p_tot, sums, onec_sb)
            tot_sb = small.tile([BPC, 1], F32)
            nc.scalar.activation(out=tot_sb, in_=p_tot, func=ACT.Identity)
            rec_sb = small.tile([BPC, 1], F32)
            nc.vector.reciprocal(rec_sb, tot_sb)
            # broadcast recip_b to the 64 output rows (row r -> b = r//16)
            p_r64 = psum.tile([BPC * NCHUNK, 1], F32)
            nc.tensor.matmul(p_r64, id_sb[0:BPC, 128:192], rec_sb)
            rec64 = small.tile([BPC * NCHUNK, 1], F32)
            nc.scalar.activation(out=rec64, in_=p_r64, func=ACT.Identity)

            # normalize with the per-partition scale port and store
            out_sb = small.tile([BPC * NCHUNK, 128], F32)
            nc.scalar.activation(out=out_sb, in_=out_unn, func=ACT.Identity,
                                 scale=rec64)
            nc.sync.dma_start(out=out, in_=out_sb)

    nc.finalize()
    return nc


def _get_nc():
    if "nc" not in _NC_CACHE:
        _NC_CACHE["nc"] = build_nc()
    return _NC_CACHE["nc"]


def make_in_maps(dec_hidden, enc_outputs, W, b):
    f32 = np.float32
    w_enc = np.asarray(W[0, DH:], dtype=f32)
    w3 = np.ascontiguousarray(
        np.broadcast_to(w_enc, (128, BPC - 1, EH2)).astype(f32))
    ident = np.zeros((128, 192), dtype=f32)
    ident[:, :128] = np.eye(128, dtype=f32)
    for b_ in range(BPC):                   # G4[b, m] = (m // NCHUNK == b)
        ident[b_, 128 + b_ * NCHUNK:128 + (b_ + 1) * NCHUNK] = 1.0
    w_dec = np.asarray(W[0, :DH], dtype=f32)
    bias = np.float32(b[0])
    # dec_contrib[b] = dec_hidden[b]·w_dec + bias (input marshaling, tiny)
    dec_c = (np.asarray(dec_hidden, dtype=f32) @ w_dec + bias).astype(f32)
    in_maps = []
    for i in range(NCORES):
        dbc = np.ascontiguousarray(
            np.broadcast_to(dec_c[i * BPC:(i + 1) * BPC], (128, BPC)))
        in_maps.append({
            "enc": np.ascontiguousarray(
                enc_outputs[:, i * BPC:(i + 1) * BPC, :].astype(f32)),
            "w3": w3,
            "dbc": dbc,
            "ident": ident,
        })
    return in_maps


def assemble_output(results):
    return np.concatenate(
        [r["out"].reshape(BPC, SRC) for r in results], axis=0).astype(np.float32)


def kernel(dec_hidden, enc_outputs, W, b):
    nc = _get_nc()
    in_maps = make_in_maps(dec_hidden, enc_outputs, W, b)
    res = run_bass_kernel_spmd(nc, in_maps, core_ids=list(range(NCORES)))
    return assemble_output(res.results)
